# revision 1
# baseline (speedup 1.0000x reference)
"""Trainium2 Bass kernel for nn_DataAugmentation (flip + resized-crop +
brightness/contrast/saturation/hue) — 8-core data-parallel.

Self-contained: takes FULL inputs, shards batch across 8 NeuronCores,
runs one Bass/Tile program per core via run_bass_kernel_spmd, gathers.
"""

import numpy as np

import concourse.bass as bass
import concourse.bacc as bacc
import concourse.tile as tile
import concourse.mybir as mybir
from concourse.bass_utils import run_bass_kernel_spmd
from concourse.dve_spec import (
    Spec, Src0, Src1, C0, C1, C2, Zero, One, maxx, minn, select, Bin, AluOp,
    lower,
)
from concourse import dve_ops as _dops
from concourse.dve_ops import DveOp, DveOpSpec, OPS, CUSTOM_DVE_SPECS, _SUB_OPCODE_FOR_NAME, has_src1

F32 = mybir.dt.float32
P = 128
OUT = 64
N_CORES = 8
B_FULL = 4096
B_CORE = B_FULL // N_CORES          # 512
GPAIRS = 16                         # pairs per group
NPAIR = B_CORE // 2                 # 256
NGROUP = NPAIR // GPAIRS            # 16
NFAC = 6                            # bf, cf, sf, osf, cb, hf6
GRAY_W = (0.2989, 0.587, 0.114)


# ---------------------------------------------------------------- custom ops
def _register_op(name, spec):
    if name in _SUB_OPCODE_FOR_NAME:
        for o in OPS:
            if o.name == name:
                return o
    opc = 1 + len(OPS)
    _SUB_OPCODE_FOR_NAME[name] = opc
    shas = {}
    for ver in ("v3", "v4"):
        try:
            s = DveOpSpec(name=name, opcode=opc, uops=lower(spec, ver=ver),
                          rd1_en=has_src1(spec))
            shas[ver] = s.sha(ver)
        except ValueError:
            pass
    op = DveOp(name, spec, subdim=False, uops_sha=shas)
    OPS.append(op)
    CUSTOM_DVE_SPECS[name] = spec
    return op


def _refbc(v, like):
    """Broadcast a [P,1] per-partition scalar (or python float) over `like`."""
    if isinstance(v, np.ndarray) and v.ndim >= 1:
        return v.reshape(v.shape[0], *([1] * (like.ndim - 1))).astype(np.float32)
    return np.float32(v)


def _refsame(v, like):
    """Reshape/broadcast an in1 operand to in0's shape."""
    if v.shape == like.shape:
        return v
    if v.size == like.size:
        return v.reshape(like.shape)
    return np.broadcast_to(v.reshape(v.shape[0], 1, -1) if v.ndim == 2 else v, like.shape)


def _absd(a, b):
    return Bin(AluOp.ABSOLUTE_DIFF, a, b)


# hat(x) = relu(1 - |x - c|): bilinear interp row weight
HAT = _register_op("AUG_HAT", Spec(
    body=maxx(One - _absd(Src0, C0), Zero),
    reference=lambda in0, in1, s0, s1, imm2:
        np.maximum(1.0 - np.abs(in0 - _refbc(s0, in0)), 0.0).astype(np.float32),
))
# tri(z) = min(|z-c1|, |z-c2|)
TRI = _register_op("AUG_TRI", Spec(
    body=minn(_absd(Src0, C0), _absd(Src0, C1)),
    reference=lambda in0, in1, s0, s1, imm2:
        np.minimum(np.abs(in0 - _refbc(s0, in0)), np.abs(in0 - _refbc(s1, in0))).astype(np.float32),
))
# qw = cr * clamp01(c0 - tri)
QW = _register_op("AUG_QW", Spec(
    body=Src1 * minn(maxx(Bin(AluOp.SUBTRACT, C0, Src0), Zero), One),
    reference=lambda in0, in1, s0, s1, imm2:
        (_refsame(in1, in0) * np.minimum(np.maximum(_refbc(s0, in0) - in0, 0.0), 1.0)).astype(np.float32),
))
# zb = mr ? 0 : (mg ? c2 : 2*c2)
ZB0 = _register_op("AUG_ZB0", Spec(
    body=select(Src0, Zero, select(Src1, C2, C2 + C2)),
    reference=lambda in0, in1, s0, s1, imm2:
        np.where(in0 != 0, 0.0, np.where(in1 != 0, imm2, 2 * imm2)).astype(np.float32),
))
# g2 = in0*c0 + in1*c1 (grayscale partial)
G2 = _register_op("AUG_G2", Spec(
    body=Src0 * C0 + Src1 * C1,
    reference=lambda in0, in1, s0, s1, imm2:
        (in0 * _refbc(s0, in0) + _refsame(in1, in0) * np.float32(s1)).astype(np.float32),
))
# cre1 = max(|in0|, |in1|); cre2 = max(in0, |in1|) + c0
CRE1 = _register_op("AUG_CRE1", Spec(
    body=maxx(maxx(Src0, Zero - Src0), maxx(Src1, Zero - Src1)),
    reference=lambda in0, in1, s0, s1, imm2:
        np.maximum(np.abs(in0), np.abs(_refsame(in1, in0))).astype(np.float32),
))
CRE2 = _register_op("AUG_CRE2", Spec(
    body=maxx(Src0, maxx(Src1, Zero - Src1)) + C0,
    reference=lambda in0, in1, s0, s1, imm2:
        (np.maximum(in0, np.abs(_refsame(in1, in0))) + np.float32(s0)).astype(np.float32),
))
# satcl = clamp01(in0*c0 + in1)   (in1 may be broadcast-shaped)
SATCL = _register_op("AUG_SATCL", Spec(
    body=minn(maxx(Src0 * C0 + Src1, Zero), One),
    reference=lambda in0, in1, s0, s1, imm2:
        np.clip(in0 * _refbc(s0, in0) + _refsame(in1, in0), 0.0, 1.0).astype(np.float32),
))


# ---------------------------------------------------------------- device program
def build_nc(b_core=B_CORE, gpairs=GPAIRS, debug=False):
    npair = b_core // 2
    ngroup = npair // gpairs
    assert ngroup * gpairs == npair
    G = gpairs
    FDP = OUT * G          # pixel-class free size per group
    AluT = mybir.AluOpType
    Act = mybir.ActivationFunctionType

    nc = bacc.Bacc("TRN2", target_bir_lowering=False, debug=debug)

    x_in = nc.dram_tensor("x_in", [b_core, 3, OUT, OUT], F32, kind="ExternalInput")
    ysv_in = nc.dram_tensor("ysv", [b_core, OUT], F32, kind="ExternalInput")
    xsv_in = nc.dram_tensor("xsv", [b_core, OUT], F32, kind="ExternalInput")
    fac_in = nc.dram_tensor("fac", [ngroup, P, NFAC * G], F32, kind="ExternalInput")
    iota_in = nc.dram_tensor("iota", [P, 2], F32, kind="ExternalInput")
    o2_in = nc.dram_tensor("o2", [P, P], F32, kind="ExternalInput")
    out_d = nc.dram_tensor("out", [b_core, 3, OUT, OUT], F32, kind="ExternalOutput")

    with tile.TileContext(nc) as tc:
        with tc.tile_pool(name="persist", bufs=1) as pers, \
             tc.tile_pool(name="grp", bufs=2) as grp, \
             tc.tile_pool(name="hue", bufs=2) as hue, \
             tc.tile_pool(name="pp", bufs=2, space="PSUM") as pp:

            IOTA = pers.tile([P, 2], F32)
            O2 = pers.tile([P, P], F32)
            nc.sync.dma_start(IOTA[:], iota_in[:])
            nc.sync.dma_start(O2[:], o2_in[:])

            imgd = [pers.tile([P, 384 * G], F32, tag=f"imgd{i}", name=f"imgd{i}") for i in range(2)]
            ryd = [pers.tile([P, 128 * G], F32, tag=f"ryd{i}", name=f"ryd{i}") for i in range(2)]
            for t in imgd + ryd:
                nc.gpsimd.memset(t[:], 0.0)

            xev = x_in[:].rearrange("(q s) c y n -> q s c y n", s=2)
            oev = out_d[:].rearrange("(q s) c y n -> q s c y n", s=2)
            yv = ysv_in[:].rearrange("(q s) k -> q s k", s=2)
            xv = xsv_in[:].rearrange("(q s) k -> q s k", s=2)

            for g in range(ngroup):
                eo = g & 1
                p0 = g * G
                img = imgd[eo]
                ry = ryd[eo]
                imgr = img[:].rearrange("p (gg c k) -> p gg c k", c=3, k=128)
                ryr = ry[:].rearrange("p (gg k) -> p gg k", k=128)

                # ---- input DMAs
                for s in range(2):   # A-half rows 0:64 / B-half 64:128
                    r0, r1 = (0, 64) if s == 0 else (64, 128)
                    c0, c1 = (0, 64) if s == 0 else (64, 128)
                    for c in range(3):
                        nc.sync.dma_start(
                            imgr[r0:r1, :, c, c0:c1],
                            xev[p0:p0 + G, s, c].transpose([1, 0, 2]))
                    nc.sync.dma_start(
                        ryr[r0:r1, :, c0:c1],
                        yv[p0:p0 + G, s, :].unsqueeze(0).broadcast_to((64, G, OUT)))

                rx = grp.tile([P, OUT * G], F32, tag="rx")
                rxr = rx[:].rearrange("p (gg k) -> p gg k", k=OUT)
                for s in range(2):
                    r0, r1 = (0, 64) if s == 0 else (64, 128)
                    nc.sync.dma_start(
                        rxr[r0:r1, :, :],
                        xv[p0:p0 + G, s, :].unsqueeze(0).broadcast_to((64, G, OUT)))

                FAC = grp.tile([P, NFAC * G], F32, tag="fac")
                nc.sync.dma_start(FAC[:], fac_in[g])
                facr = FAC[:].rearrange("p (gg s) -> p gg s", s=NFAC)

                # ---- build interp weight matrices in place (hat of |k - pos|)
                # positions and iota are shifted by +2 so hat(0 - k) == 0 on the
                # zero off-diag blocks; run full-partition on ACT (partition-
                # offset custom ops are a silent no-op on HW; ACT has slack).
                for t_ap in (ry[:], rx[:]):
                    nc.scalar.activation(t_ap, t_ap, Act.Abs, bias=IOTA[:, 1:2])
                    nc.scalar.activation(t_ap, t_ap, Act.Relu, bias=1.0, scale=-1.0)

                # ---- per-pair resize matmuls + brightness
                xbuf = grp.tile([P, 192 * G], F32, tag="xbuf")
                for p in range(G):
                    T1 = pp.tile([P, 384], F32, tag="t1", bufs=3)
                    for c in range(3):
                        nc.tensor.matmul(T1[:, 128 * c:128 * (c + 1)],
                                         imgr[:, p, c, :], ryr[:, p, :],
                                         start=True, stop=True)
                    sbt = grp.tile([P, 384], F32, tag="sbt", bufs=4)
                    nc.scalar.copy(sbt[:], T1[:])
                    T2 = pp.tile([P, 192], F32, tag="t2", bufs=3)
                    for c in range(3):
                        nc.tensor.matmul(T2[:, 64 * c:64 * (c + 1)],
                                         sbt[:, 128 * c:128 * (c + 1)],
                                         rxr[:, p, :], start=True, stop=True)
                    # brightness: x1 = min(x0 * bf, 1)  (PSUM -> SBUF)
                    nc.vector.tensor_scalar(
                        xbuf[:, 192 * p:192 * (p + 1)], T2[:],
                        facr[:, p, 0:1], 1.0, AluT.mult, AluT.min)

                xr = xbuf[:].rearrange("p (gg c k) -> p gg c k", c=3, k=OUT)

                # ---- contrast mean: gray1 of x1, per-sample sums
                gray = grp.tile([P, FDP], F32, tag="gray")
                grayr = gray[:].rearrange("p (gg k) -> p gg k", k=OUT)
                nc.vector._custom_dve(G2, out=grayr[:, :, :], in0=xr[:, :, 0, :],
                                      in1=xr[:, :, 1, :], s0=GRAY_W[0], s1=GRAY_W[1])
                nc.vector.scalar_tensor_tensor(
                    grayr[:, :, :], xr[:, :, 2, :], GRAY_W[2], grayr[:, :, :],
                    AluT.mult, AluT.add)
                mrow = grp.tile([P, G], F32, tag="mrow")
                tri = hue.tile([P, FDP], F32, tag="tri")
                for p in range(G):
                    nc.scalar.activation(tri[:, OUT * p:OUT * (p + 1)],
                                         grayr[:, p, :], Act.Copy,
                                         accum_out=mrow[:, p:p + 1])
                Mcol = pp.tile([P, G], F32, tag="mcol")
                nc.tensor.matmul(Mcol[:], O2[:], mrow[:], start=True, stop=True)
                tb8 = grp.tile([P, G], F32, tag="tb8")
                nc.vector.tensor_tensor(tb8[:], Mcol[:], facr[:, :, 4], AluT.mult)

                # ---- contrast: x2 = clamp01(cf*x1 + tb)  (in place)
                for p in range(G):
                    nc.scalar.activation(xbuf[:, 192 * p:192 * (p + 1)],
                                         xbuf[:, 192 * p:192 * (p + 1)],
                                         Act.Identity, bias=tb8[:, p:p + 1],
                                         scale=facr[:, p, 1:2])
                nc.vector.tensor_scalar(xbuf[:], xbuf[:], 0.0, 1.0, AluT.max, AluT.min)
                x2r = xr

                # ---- saturation: gray2 from clamped x2; x3 = clamp01(sf*x2 + osf*gray2)
                nc.vector._custom_dve(G2, out=grayr[:, :, :], in0=x2r[:, :, 0, :],
                                      in1=x2r[:, :, 1, :], s0=GRAY_W[0], s1=GRAY_W[1])
                nc.vector.scalar_tensor_tensor(
                    grayr[:, :, :], x2r[:, :, 2, :], GRAY_W[2], grayr[:, :, :],
                    AluT.mult, AluT.add)
                g2s = grp.tile([P, FDP], F32, tag="g2s")
                g2sr = g2s[:].rearrange("p (gg k) -> p gg k", k=OUT)
                for p in range(G):
                    nc.scalar.activation(g2sr[:, p, :], grayr[:, p, :], Act.Copy,
                                         scale=facr[:, p, 3:4])
                x3r = xr
                for p in range(G):
                    nc.vector._custom_dve(
                        SATCL, out=x3r[:, p, :, :], in0=x3r[:, p, :, :],
                        in1=g2sr[:, p, :].unsqueeze(1).broadcast_to((P, 3, OUT)),
                        s0=facr[:, p, 2:3])

                # ---- hue
                r_s = x3r[:, :, 0, :]
                g_s = x3r[:, :, 1, :]
                b_s = x3r[:, :, 2, :]
                tA = hue.tile([P, FDP], F32, tag="tA")   # mx1 / mn scratch
                mx = hue.tile([P, FDP], F32, tag="mx")
                cre = hue.tile([P, FDP], F32, tag="cre")
                rcr = hue.tile([P, FDP], F32, tag="rcr")
                esel = hue.tile([P, FDP], F32, tag="esel")
                e1 = hue.tile([P, FDP], F32, tag="e1")
                e2 = hue.tile([P, FDP], F32, tag="e2")
                mr = hue.tile([P, FDP], mybir.dt.uint8, tag="mr")
                mg = hue.tile([P, FDP], mybir.dt.uint8, tag="mg")
                tAr = tA[:].rearrange("p (gg k) -> p gg k", k=OUT)
                mxr = mx[:].rearrange("p (gg k) -> p gg k", k=OUT)

                eselr = esel[:].rearrange("p (gg k) -> p gg k", k=OUT)
                e1r = e1[:].rearrange("p (gg k) -> p gg k", k=OUT)
                e2r = e2[:].rearrange("p (gg k) -> p gg k", k=OUT)
                nc.vector.tensor_tensor(eselr[:, :, :], r_s, g_s, AluT.subtract)  # e3
                nc.vector.tensor_tensor(e1r[:, :, :], g_s, b_s, AluT.subtract)
                nc.vector.tensor_tensor(e2r[:, :, :], b_s, r_s, AluT.subtract)
                # cre = max(|e1|,|e2|,|e3|) + eps  (== mx - mn + eps)
                nc.vector._custom_dve(CRE1, out=cre[:], in0=e1[:], in1=e2[:])
                nc.vector._custom_dve(CRE2, out=cre[:], in0=cre[:], in1=esel[:], s0=1e-20)
                nc.vector.reciprocal_approx_fast(rcr[:], cre[:])
                nc.vector.tensor_tensor(tA[:], r_s, g_s, AluT.max)
                nc.vector.tensor_tensor(mxr[:, :, :], tAr[:, :, :], b_s, AluT.max)
                nc.vector.tensor_tensor(mr[:].rearrange("p (gg k) -> p gg k", k=OUT),
                                        mxr[:, :, :], r_s, AluT.is_equal)
                nc.vector.tensor_tensor(mg[:].rearrange("p (gg k) -> p gg k", k=OUT),
                                        mxr[:, :, :], g_s, AluT.is_equal)
                nc.vector.copy_predicated(esel[:], mg[:], e2[:])
                nc.vector.copy_predicated(esel[:], mr[:], e1[:])
                # zb -> e2 tile (reuse); h6 -> e1 tile (reuse); z -> esel
                nc.vector._custom_dve(ZB0, out=e2[:], in0=mr[:], in1=mg[:], imm2=2.0)
                nc.vector.tensor_tensor(e1[:], esel[:], rcr[:], AluT.mult)
                nc.vector.tensor_tensor(esel[:], e1[:], e2[:], AluT.add)
                zt = esel
                ztr = zt[:].rearrange("p (gg k) -> p gg k", k=OUT)
                nc.vector.tensor_tensor(
                    ztr[:, :, :], ztr[:, :, :],
                    facr[:, :, 5].unsqueeze(2).broadcast_to((P, G, OUT)), AluT.add)
                # out_n = mx - cr * trap(z), n = 5(r), 3(g), 1(b)
                for ci, (cc1, cc2) in enumerate(((-3.0, 3.0), (-1.0, 5.0), (1.0, 7.0))):
                    nc.vector._custom_dve(TRI, out=tri[:], in0=zt[:], s0=cc1, s1=cc2)
                    nc.vector._custom_dve(QW, out=tri[:], in0=tri[:], in1=cre[:], s0=2.0)
                    nc.vector.tensor_tensor(x3r[:, :, ci, :], mxr[:, :, :],
                                            tri[:].rearrange("p (gg k) -> p gg k", k=OUT),
                                            AluT.subtract)

                # ---- output DMAs
                for s in range(2):
                    r0, r1 = (0, 64) if s == 0 else (64, 128)
                    for c in range(3):
                        nc.sync.dma_start(
                            oev[p0:p0 + G, s, c].transpose([1, 0, 2]),
                            x3r[r0:r1, :, c, :])

    nc.compile()
    return nc


# ---------------------------------------------------------------- host prep
def host_prep(x, flip_mask, crop_i, crop_j, crop_h, crop_w,
              b_factor, c_factor, s_factor, h_factor,
              b_core=B_CORE, gpairs=GPAIRS):
    f32 = np.float32
    B = x.shape[0]
    npair = b_core // 2
    ngroup = npair // gpairs
    G = gpairs

    ar = (np.arange(OUT, dtype=f32) + f32(0.5))
    ys = crop_i[:, None].astype(f32) + ar[None, :] * (crop_h.astype(f32)[:, None] / f32(OUT)) - f32(0.5)
    xs = crop_j[:, None].astype(f32) + ar[None, :] * (crop_w.astype(f32)[:, None] / f32(OUT)) - f32(0.5)

    def eff(p):
        return np.where(p < 0, p + f32(1.0), np.minimum(p, f32(63.0))).astype(f32)

    ysv = (eff(ys) + f32(2.0)).astype(f32)
    xsv = (np.where(flip_mask[:, None], f32(63.0) - eff(xs), eff(xs)) + f32(2.0)).astype(f32)

    bf = b_factor.astype(f32)
    cf = c_factor.astype(f32)
    sf = s_factor.astype(f32)
    osf = (f32(1.0) - sf).astype(f32)
    cb = ((f32(1.0) - cf) / f32(OUT * OUT * 1.0)).astype(f32) / f32(1.0)
    cb = ((f32(1.0) - cf) / f32(4096.0)).astype(f32)
    hf6 = (f32(6.0) * h_factor.astype(f32)).astype(f32)

    kk = np.concatenate([np.arange(64, dtype=f32)] * 2) + f32(2.0)
    iota = np.stack([kk, -kk], axis=1).astype(f32)
    o2 = np.zeros((P, P), dtype=f32)
    o2[:64, :64] = 1.0
    o2[64:, 64:] = 1.0

    per_core = []
    n_cores = B // b_core
    for k in range(n_cores):
        sl = slice(k * b_core, (k + 1) * b_core)
        fac = np.zeros((ngroup, P, NFAC * G), dtype=f32)
        vals = np.stack([bf[sl], cf[sl], sf[sl], osf[sl], cb[sl], hf6[sl]], -1)  # [b_core, 6]
        vals = vals.reshape(ngroup, G, 2, NFAC)
        for s, rows in ((0, slice(0, 64)), (1, slice(64, 128))):
            v = vals[:, :, s, :].reshape(ngroup, 1, G * NFAC)
            fac[:, rows, :] = np.broadcast_to(v, (ngroup, 64, G * NFAC))
        per_core.append({
            "x_in": np.ascontiguousarray(x[sl].astype(f32)),
            "ysv": np.ascontiguousarray(ysv[sl]),
            "xsv": np.ascontiguousarray(xsv[sl]),
            "fac": np.ascontiguousarray(fac),
            "iota": iota,
            "o2": o2,
        })
    return per_core


_NC_CACHE = {}


def kernel(**inputs):
    x = np.asarray(inputs["x"], dtype=np.float32)
    args = {k: np.asarray(inputs[k]) for k in
            ("flip_mask", "crop_i", "crop_j", "crop_h", "crop_w",
             "b_factor", "c_factor", "s_factor", "h_factor")}
    in_maps = host_prep(x, args["flip_mask"], args["crop_i"], args["crop_j"],
                        args["crop_h"], args["crop_w"], args["b_factor"],
                        args["c_factor"], args["s_factor"], args["h_factor"])
    key = (B_CORE, GPAIRS)
    if key not in _NC_CACHE:
        _NC_CACHE[key] = build_nc(B_CORE, GPAIRS)
    nc = _NC_CACHE[key]
    res = run_bass_kernel_spmd(nc, in_maps, list(range(N_CORES)))
    outs = [np.asarray(r["out"]) for r in res.results]
    return np.concatenate(outs, axis=0).astype(np.float32)


if __name__ == "__main__":
    nc = build_nc()
    print("built ok")

